# revision 19
# baseline (speedup 1.0000x reference)
"""Tensor-parallel causal attention layer (RoPE) for 8 Trainium2 NeuronCores.

Problem: nn_AttentionTier (B=4, T=2048, D=1024, H=16, Dh=64), fp32 I/O.

Sharding: DP=4 over batch x TP=2 over heads (8 heads per core).
  core c -> batch c//2, head group c%2 (heads 8*(c%2) .. 8*(c%2)+8).

v5.1 — v4's bf16 compute (fp8 DoubleRow projections measured slower on
HW: LDWEIGHTS doesn't register as PE activity, so the HAM clock-gate
throttles through the low-duty DR chains) plus:
  - single-descriptor 3D input DMAs and a two-queue preamble so the
    first matmul starts ~4x earlier.
  - global deferred-work queue: projection chains of block tb+1 AND the
    out-projection of block qb-1 drip into attention gaps of block qb
    with stride pacing, so the PE stays fed through the last block.
  - emission-before-use enforced via ensure() keys (same-engine program
    order is a dependency order).
  - ACT runs only exp + the rowsum recip (ln/exp); every PSUM
    evacuation is on DVE; rope cos-mult and rope-add on GPSIMD.

Per core:
  - Attention: per k-chunk the two heads of a plane get score matmuls on
    disjoint PE row groups (tile_position (0,0)/(64,0), concurrent); one
    exp per chunk covers both heads; ones-augmented V gives row sums.
  - Softmax: rowsums gathered to partitions {0,32,64,96}, one ln+exp(-x)
    pass, reciprocal broadcast via K=1 matmuls on packed row groups.
  - Out-projection partials sharded by TOKEN half; pairwise bf16
    ReduceScatter; stored bf16 and upcast to fp32 on the host.
"""

import sys

sys.path.insert(0, "/opt/trn_rl_repo")

import numpy as np

B, T, D = 4, 2048, 1024
H, Dh = 16, 64
N_CORES = 8
P = 128
TB = 512          # token block (matmul moving dim)
NTB = T // TB     # 4
NCC = D // P      # 8 contraction chunks
HLOC = H // 2     # heads per core

_CACHE = {}


def _patch_act_tables():
    """Force every ACT function we use into one table set so bacc emits a
    single hoisted InstLoadActFuncSet instead of thrashing between the
    exp- and ln-anchored sets on every softmax row."""
    import functools
    import concourse.mybir as mybir
    from concourse import bacc, hw_specs

    if getattr(bacc.get_activation_tables, "_attn_patched", False):
        return
    orig = hw_specs.get_activation_tables
    AF = mybir.ActivationFunctionType
    ours = {AF.Exp, AF.Ln, AF.Copy, AF.Identity}

    @functools.cache
    def patched(module_arch):
        tabs = dict(orig(module_arch))
        return {
            name: (fns if name == "natural_log_exp_and_others"
                   else set(fns) - ours)
            for name, fns in tabs.items()
        }

    patched._attn_patched = True
    bacc.get_activation_tables = patched


def _build_program(reps=1):
    import concourse.bass as bass  # noqa: F401
    import concourse.mybir as mybir
    import concourse.tile as tile
    from concourse import bacc

    _patch_act_tables()

    f32 = mybir.dt.float32
    bf16 = mybir.dt.bfloat16
    fp8 = mybir.dt.float8e4
    AF = mybir.ActivationFunctionType
    DR = mybir.MatmulPerfMode.DoubleRow

    nc = bacc.Bacc("TRN2", target_bir_lowering=False, debug=False,
                   num_devices=N_CORES)

    # ---- DRAM I/O ----
    xT_d = nc.dram_tensor("xT", [D, T], bf16, kind="ExternalInput").ap()
    wqkT_d = nc.dram_tensor("wqkT", [D, D], bf16, kind="ExternalInput").ap()
    wvT_d = nc.dram_tensor("wvT", [D, D // 2], bf16,
                           kind="ExternalInput").ap()
    woutT_d = nc.dram_tensor("woutT", [D // 2, D], bf16,
                             kind="ExternalInput").ap()
    r2T_d = nc.dram_tensor("r2T", [P, P], bf16, kind="ExternalInput").ap()
    cos2_d = nc.dram_tensor("cos2", [P, T], bf16, kind="ExternalInput").ap()
    sin2_d = nc.dram_tensor("sin2", [P, T], bf16, kind="ExternalInput").ap()
    tri_d = nc.dram_tensor("tri", [P, P], bf16, kind="ExternalInput").ap()
    out_d = nc.dram_tensor("out", [D // 2, T], bf16,
                           kind="ExternalOutput").ap()

    groups = [[0, 1], [2, 3], [4, 5], [6, 7]]
    ESC = 0.125 / 1024.0  # 1/sqrt(Dh) / (32*32 W_qk host prescale)

    with tile.TileContext(nc) as tc:
        with tc.tile_pool(name="const", bufs=1) as constp, \
             tc.tile_pool(name="big", bufs=1) as bigp, \
             tc.tile_pool(name="w1", bufs=1) as w1p, \
             tc.tile_pool(name="ph1", bufs=3) as ph1, \
             tc.tile_pool(name="xtp", bufs=2) as xtp, \
             tc.tile_pool(name="att", bufs=6) as attp, \
             tc.tile_pool(name="msc", bufs=2) as mscp, \
             tc.tile_pool(name="aop", bufs=2) as aop, \
             tc.tile_pool(name="dram", bufs=2, space="DRAM") as dramp, \
             tc.tile_pool(name="ps_s", bufs=2, space="PSUM") as ps_s, \
             tc.tile_pool(name="ps_o", bufs=1, space="PSUM") as ps_o, \
             tc.tile_pool(name="ps_x", bufs=2, space="PSUM") as ps_x:

            r2T = constp.tile([P, P], bf16)
            tri = constp.tile([P, P], bf16)
            ones_b = constp.tile([P, P], bf16)
            nc.vector.memset(ones_b[:], 1.0)

            # persistent big tensors (bf16)
            qk = bigp.tile([P, NCC, T], bf16)              # rope'd q^T,k^T
            vbar = bigp.tile([P, T // P, HLOC, Dh + 1], bf16)
            nc.vector.tensor_copy(
                vbar[:, :, :, Dh:Dh + 1],
                ones_b[:, None, :HLOC, None].to_broadcast(
                    [P, T // P, HLOC, 1]))

            wqkT = w1p.tile([P, NCC, D], bf16)
            wvT = w1p.tile([P, NCC, D // 2], bf16)
            woutT = w1p.tile([P, NCC // 2, D], bf16)

            # per-chunk descriptors so transfers spread across the 8
            # parallel DMA hardware queues (a single big descriptor
            # serializes on one queue at ~30 GB/s)
            def load_xT(tb):
                t = xtp.tile([P, NCC, TB], bf16, tag="xT")
                for cc in range(NCC):
                    nc.gpsimd.dma_start(
                        t[:, cc],
                        xT_d[cc * P:(cc + 1) * P, tb * TB:(tb + 1) * TB])
                return t

            # preamble: QK-proj inputs first, on two issue queues
            xTs = {0: load_xT(0)}
            for cc in range(NCC):
                nc.sync.dma_start(wqkT[:, cc], wqkT_d[cc * P:(cc + 1) * P, :])
            nc.sync.dma_start(r2T[:], r2T_d[:])
            nc.sync.dma_start(tri[:], tri_d[:])
            for cc in range(NCC):
                nc.gpsimd.dma_start(wvT[:, cc], wvT_d[cc * P:(cc + 1) * P, :])

            # ---- emission helpers ----
            cstiles = {}

            def cs_tiles(tb):
                if tb not in cstiles:
                    tsl = slice(tb * TB, (tb + 1) * TB)
                    cosb = ph1.tile([P, TB], bf16, tag="cosb",
                                    name=f"cosb_{tb}")
                    sinb = ph1.tile([P, TB], bf16, tag="sinb",
                                    name=f"sinb_{tb}")
                    nc.sync.dma_start(cosb[:], cos2_d[:, tsl])
                    nc.sync.dma_start(sinb[:], sin2_d[:, tsl])
                    cstiles[tb] = (cosb, sinb)
                return cstiles[tb]

            def proj_qk(tb, oc):
                """one 128-feature chunk of QK projection + rope"""
                tsl = slice(tb * TB, (tb + 1) * TB)
                xT = xTs[tb]
                cosb, sinb = cs_tiles(tb)
                qk_ps = ps_x.tile([P, TB], f32, tag="xps",
                                  name=f"qkps_{tb}_{oc}")
                for cc in range(NCC):
                    nc.tensor.matmul(
                        qk_ps[:], wqkT[:, cc, oc * P:(oc + 1) * P],
                        xT[:, cc, :],
                        start=(cc == 0), stop=(cc == NCC - 1))
                raw = ph1.tile([P, TB], bf16, tag="raw",
                               name=f"raw_{tb}_{oc}")
                nc.vector.tensor_copy(raw[:], qk_ps[:])
                rot_ps = ps_x.tile([P, TB], f32, tag="xps",
                                   name=f"rotps_{tb}_{oc}")
                nc.tensor.matmul(rot_ps[:], r2T[:], raw[:],
                                 start=True, stop=True)
                ta = ph1.tile([P, TB], bf16, tag="ta", name=f"ta_{tb}_{oc}")
                nc.vector.tensor_tensor(
                    ta[:], rot_ps[:], sinb[:], mybir.AluOpType.mult)
                tb_ = ph1.tile([P, TB], bf16, tag="tb_",
                               name=f"tb__{tb}_{oc}")
                nc.gpsimd.tensor_tensor(
                    tb_[:], raw[:], cosb[:], mybir.AluOpType.mult)
                nc.gpsimd.tensor_tensor(
                    qk[:, oc, tsl], ta[:], tb_[:], mybir.AluOpType.add)

            def proj_v(tb, ts):
                """one 128-token chunk of the V projection"""
                xT = xTs[tb]
                v_ps = ps_x.tile([P, D // 2], f32, tag="xps",
                                 name=f"vps_{tb}_{ts}")
                for cc in range(NCC):
                    nc.tensor.matmul(
                        v_ps[:], xT[:, cc, ts * P:(ts + 1) * P],
                        wvT[:, cc, :],
                        start=(cc == 0), stop=(cc == NCC - 1))
                tc_idx = tb * (TB // P) + ts
                nc.vector.tensor_copy(
                    vbar[:, tc_idx, :, 0:Dh],
                    v_ps[:].rearrange("p (h d) -> p h d", h=HLOC))

            # ---- deferred-work queue -------------------------------
            # pending maps key -> emission thunk. ensure() force-emits
            # (consumers must be emitted after producers on each engine
            # queue); filler() drip-feeds chains into attention gaps.
            pending = {}
            order = []

            def enq(key, thunk):
                pending[key] = thunk
                order.append(key)

            def ensure(key):
                t = pending.pop(key, None)
                if t is not None:
                    t()

            def pop_one():
                while order:
                    key = order[0]
                    if key not in pending:
                        order.pop(0)
                        continue
                    order.pop(0)
                    pending.pop(key)()
                    return True
                return False

            def flush():
                while pop_one():
                    pass

            qstate = {}

            def attn_state(qb):
                if qb not in qstate:
                    sstack = mscp.tile([P, 2, TB], bf16, tag="sstack",
                                       name=f"sstack_{qb}")
                    rstack = mscp.tile([P, 2, TB], bf16, tag="rstack",
                                       name=f"rstack_{qb}")
                    qstate[qb] = (sstack, rstack, [None] * (HLOC // 2))
                return qstate[qb]

            def attn_hp(qb, hp, pace=None, fill=True):
                """attention for head pair hp of q block qb + rowsum
                gather; deferred chains drip in between chunks, paced by a
                fractional credit so the qb-cycle inventory spreads evenly
                over all its chunks."""
                sstack, rstack, osbs = attn_state(qb)
                qsl = slice(qb * TB, (qb + 1) * TB)
                h1, h2 = 2 * hp, 2 * hp + 1
                kpl = NCC // 2 + hp
                qpl = hp
                ensure(("qk", qb, qpl))
                ensure(("qk", qb, kpl))

                def filler():
                    if pace is None:
                        return
                    pace["credit"] += pace["ppc"]
                    while pace["credit"] >= 1.0:
                        pace["credit"] -= 1.0
                        was_op = order and order[0][0] == "op"
                        if not pop_one():
                            pace["credit"] = 0.0
                            break
                        if was_op and order and order[0][0] == "op":
                            pop_one()

                def smm(s_pair, kc, fsl):
                    ks = slice(kc * P, (kc + 1) * P)
                    nc.tensor.matmul(
                        s_pair[:, 0, fsl],
                        qk[0:Dh, kpl, ks], qk[0:Dh, qpl, qsl][:, fsl],
                        start=True, stop=True, tile_position=(0, 0))
                    nc.tensor.matmul(
                        s_pair[:, 1, fsl],
                        qk[Dh:P, kpl, ks], qk[Dh:P, qpl, qsl][:, fsl],
                        start=True, stop=True, tile_position=(64, 0))

                o_pair = ps_o.tile([Dh + 1, 2, TB], f32, tag="ops",
                                   name=f"op_{qb}_{hp}")
                for kc in range(4 * qb):
                    ensure(("v", kc // 4, kc % 4))
                    s_pair = ps_s.tile([P, 2, TB], f32, tag="sps",
                                       name=f"sp_{qb}_{hp}_{kc}")
                    smm(s_pair, kc, slice(0, TB))
                    pt = attp.tile([P, 2, TB], bf16, tag="pt",
                                   name=f"pt_{qb}_{hp}_{kc}")
                    nc.scalar.activation(
                        pt[:], s_pair[:], AF.Exp, scale=0.125)
                    for j, h in ((0, h1), (1, h2)):
                        nc.tensor.matmul(
                            o_pair[:, j, :], vbar[:, kc, h, :], pt[:, j, :],
                            start=(kc == 0), stop=False,
                            skip_group_check=True)
                    if fill:
                        filler()
                for cr in range(4):
                    kc = 4 * qb + cr
                    ensure(("v", kc // 4, kc % 4))
                    qo = cr * P
                    fsl = slice(qo, TB)
                    s_pair = ps_s.tile([P, 2, TB], f32, tag="sps",
                                       name=f"spd_{qb}_{hp}_{cr}")
                    smm(s_pair, kc, fsl)
                    pt = attp.tile([P, 2, TB], bf16, tag="pt",
                                   name=f"ptd_{qb}_{hp}_{cr}")
                    nc.scalar.activation(
                        pt[:, :, fsl], s_pair[:, :, fsl], AF.Exp,
                        scale=0.125)
                    nc.vector.tensor_tensor(
                        pt[:, :, qo:qo + P], pt[:, :, qo:qo + P],
                        tri[:, None, :].to_broadcast([P, 2, P]),
                        mybir.AluOpType.mult)
                    for j, h in ((0, h1), (1, h2)):
                        nc.tensor.matmul(
                            o_pair[:, j, fsl], vbar[:, kc, h, :],
                            pt[:, j, fsl],
                            start=(kc == 0), stop=(cr == 3),
                            skip_group_check=True)
                    if fill:
                        filler()

                # evacuate PSUM; one DMA gathers both heads' rowsum rows to
                # partition 32*hp of sstack
                osb = mscp.tile([Dh + 1, 2, TB], bf16, tag=f"osb{hp % 2}",
                                name=f"osb_{qb}_{hp}")
                nc.vector.tensor_copy(osb[:], o_pair[:])
                nc.gpsimd.dma_start(
                    sstack[32 * hp:32 * hp + 1, :, :], osb[Dh:Dh + 1, :, :])
                osbs[hp] = osb

            def norm_recip(qb):
                """batched reciprocal of all 8 rowsums: 1/x = exp(-ln x)"""
                sstack, rstack, _ = attn_state(qb)
                nc.scalar.activation(sstack[:], sstack[:], AF.Ln)
                nc.scalar.activation(rstack[:], sstack[:], AF.Exp,
                                     scale=-1.0)

            def norm_apply(qb, aout):
                _, rstack, osbs = attn_state(qb)
                for h in range(HLOC):
                    hp, j = h // 2, h % 2
                    rp = 32 * hp
                    b_ps = ps_x.tile([Dh, TB], f32, tag="xps",
                                     name=f"bps_{qb}_{h}")
                    nc.tensor.matmul(
                        b_ps[:], ones_b[rp:rp + 1, 0:Dh],
                        rstack[rp:rp + 1, j, :],
                        start=True, stop=True, tile_position=(rp, 0))
                    nc.vector.tensor_tensor(
                        aout[64 * j:64 * j + Dh, hp, :],
                        osbs[hp][0:Dh, j, :], b_ps[:],
                        mybir.AluOpType.mult)

            def enq_outproj(qb, aout):
                """queue the out-projection of block qb as per-feature-chunk
                chains; each group's pairwise ReduceScatter fires with its
                4th chain. Core at pair-rank r ends with global output
                features r*512..r*512+512 for all tokens."""
                parts = {}
                for g in (0, 1):
                    parts[g] = dramp.tile([D // 2, TB], bf16, tag="part",
                                          name=f"part_{qb}_{g}")

                def ec_chain(g, i, ec):
                    part = parts[g]
                    f_ps = ps_x.tile([P, TB], f32, tag="xps",
                                     name=f"fps_{qb}_{g}_{ec}")
                    for cc in range(NCC // 2):
                        nc.tensor.matmul(
                            f_ps[:], woutT[:, cc, ec * P:(ec + 1) * P],
                            aout[:, cc, :],
                            start=(cc == 0), stop=(cc == NCC // 2 - 1))
                    fsb = mscp.tile([P, TB], bf16, tag="fsb",
                                    name=f"fsb_{qb}_{g}_{ec}")
                    nc.vector.tensor_copy(fsb[:], f_ps[:])
                    row = ((0 if ec < 4 else 256)
                           + (ec - (0 if ec < 4 else 4) - 2 * g) * P)
                    nc.sync.dma_start(part[row:row + P, :], fsb[:])
                    if i == 3:
                        rs_o = dramp.tile([D // 4, TB], bf16, tag="rs_o",
                                          name=f"rso_{qb}_{g}")
                        nc.gpsimd.collective_compute(
                            "ReduceScatter", mybir.AluOpType.add,
                            replica_groups=groups,
                            ins=[part.opt()], outs=[rs_o.opt()])
                        nc.sync.dma_start(
                            out_d[g * (D // 4):(g + 1) * (D // 4),
                                  qb * TB:(qb + 1) * TB], rs_o[:])

                for g, ecs in ((0, (0, 1, 4, 5)), (1, (2, 3, 6, 7))):
                    for i, ec in enumerate(ecs):
                        enq(("op", qb, g, ec),
                            lambda g=g, i=i, ec=ec: ec_chain(g, i, ec))

            # ---- software-pipelined emission ----
            # block tb=0 projections up front; thereafter projections of
            # tb+1 and the out-projection of qb-1 drip into qb's attention.
            cs_tiles(0)
            for oc in (0, 4, 1, 5, 2, 6, 3, 7):
                proj_qk(0, oc)
            for ts in range(TB // P):
                proj_v(0, ts)
            xTs[1] = load_xT(1)

            # woutT queued after x(1) — not needed until the first out-proj
            for cc in range(NCC // 2):
                nc.sync.dma_start(
                    woutT[:, cc], woutT_d[cc * P:(cc + 1) * P, :])

            attn_hp(0, 0, fill=False)
            for qb in range(NTB):
                if qb + 1 < NTB:
                    ntb = qb + 1
                    for hp in range(HLOC // 2):
                        enq(("qk", ntb, hp),
                            lambda t=ntb, o=hp: proj_qk(t, o))
                        enq(("qk", ntb, NCC // 2 + hp),
                            lambda t=ntb, o=NCC // 2 + hp: proj_qk(t, o))
                        if hp == 0:
                            for ts in range(TB // P):
                                enq(("v", ntb, ts),
                                    lambda t=ntb, s=ts: proj_v(t, s))
                    if qb + 2 < NTB:
                        xTs[qb + 2] = load_xT(qb + 2)
                chunks = 3 * (4 * qb + 4)
                if qb + 1 < NTB:
                    chunks += 4 * (qb + 1) + 4
                pace = {"ppc": max(0, len(pending) - 2) / max(1, chunks),
                        "credit": 0.0}
                for hp in range(1, HLOC // 2):
                    attn_hp(qb, hp, pace=pace)
                norm_recip(qb)
                # keep the PE fed across the recip latency window
                pop_one()
                pop_one()
                if qb + 1 < NTB:
                    attn_hp(qb + 1, 0, pace=pace)
                # bound deferral: qb-1's out-proj chains must be emitted
                # before qb's are queued (aout pool is 2 deep)
                if qb > 0:
                    for g, ecs in ((0, (0, 1, 4, 5)), (1, (2, 3, 6, 7))):
                        for ec in ecs:
                            ensure(("op", qb - 1, g, ec))
                aout = aop.tile([P, NCC // 2, TB], bf16, tag="aout",
                                name=f"aout_{qb}")
                norm_apply(qb, aout)
                enq_outproj(qb, aout)
                if qb == NTB - 1:
                    flush()
            flush()

    nc.compile()
    return nc


def _host_inputs(x, W_qkv, W_out):
    """Per-core input dicts."""
    import ml_dtypes
    bf = ml_dtypes.bfloat16
    f8 = ml_dtypes.float8_e4m3
    x = np.ascontiguousarray(np.asarray(x, dtype=np.float32))
    W_qkv = np.asarray(W_qkv, dtype=np.float32)
    W_out = np.asarray(W_out, dtype=np.float32)

    def to8(a):
        return np.clip(a, -240.0, 240.0).astype(f8)

    # rope tables, transposed layout, 2-head stack
    inv = 1.0 / (10000.0 ** (np.arange(0, Dh, 2, dtype=np.float64) / Dh))
    ang = np.outer(np.arange(T, dtype=np.float64), inv)        # (T, 32)
    emb = np.concatenate([ang, ang], axis=1)                   # (T, 64)
    cosT = np.cos(emb).astype(np.float32).T                    # (64, T)
    sinT = np.sin(emb).astype(np.float32).T
    cos2 = np.ascontiguousarray(
        np.concatenate([cosT, cosT], 0)).astype(bf)            # (128, T)
    sin2 = np.ascontiguousarray(
        np.concatenate([sinT, sinT], 0)).astype(bf)

    # rotation matrix: rot(q) = R @ q ; lhsT = R2.T
    R = np.zeros((Dh, Dh), np.float32)
    for d in range(Dh // 2):
        R[d, d + Dh // 2] = -1.0
        R[d + Dh // 2, d] = 1.0
    R2 = np.zeros((P, P), np.float32)
    R2[:Dh, :Dh] = R
    R2[Dh:, Dh:] = R
    r2T = np.ascontiguousarray(R2.T).astype(bf)

    # triangular mask in scores^T layout: keep k <= q
    tri = np.triu(np.ones((P, P), np.float32)).astype(bf)

    ins = []
    for c in range(N_CORES):
        b, tp = c // 2, c % 2
        heads = range(8 * tp, 8 * tp + 8)
        wq = np.concatenate([W_qkv[64 * h: 64 * h + 64] for h in heads], 0)
        wk = np.concatenate(
            [W_qkv[D + 64 * h: D + 64 * h + 64] for h in heads], 0)
        wv = np.concatenate(
            [W_qkv[2 * D + 64 * h: 2 * D + 64 * h + 64] for h in heads], 0)
        wqkT = np.ascontiguousarray(
            np.concatenate([wq, wk], 0).T).astype(bf)               # (1024,1024)
        wvT = np.ascontiguousarray(wv.T).astype(bf)                 # (1024,512)
        woutT = np.ascontiguousarray(
            W_out[:, 512 * tp: 512 * tp + 512].T).astype(bf)        # (512,1024)
        ins.append({
            "xT": np.ascontiguousarray(x[b].T).astype(bf),
            "wqkT": wqkT, "wvT": wvT, "woutT": woutT,
            "r2T": r2T, "cos2": cos2, "sin2": sin2, "tri": tri,
        })
    return ins


def kernel(x, W_qkv, W_out):
    import time
    from concourse.bass_utils import run_bass_kernel_spmd

    if "nc" not in _CACHE:
        _CACHE["nc"] = _build_program()
    nc = _CACHE["nc"]
    ins = _host_inputs(x, W_qkv, W_out)
    res = None
    for attempt in range(3):
        try:
            res = run_bass_kernel_spmd(nc, ins, list(range(N_CORES)))
            break
        except Exception:
            # if a previous process' device teardown raced our startup the
            # first execution can die; give the worker time to come back and
            # drop any broken backend handles before retrying
            if attempt == 2:
                raise
            time.sleep(30)
            try:
                import jax
                jax.clear_caches()
                jax.clear_backends()
            except Exception:
                pass
    out = np.empty((B, T, D), dtype=np.float32)
    for c in range(N_CORES):
        b, tp = c // 2, c % 2
        o = np.asarray(res.results[c]["out"]).astype(np.float32)  # (D//2, T)
        out[b, :, 512 * tp: 512 * tp + 512] = o.T
    return out


# revision 22
# speedup vs baseline: 1.1591x; 1.1591x over previous
"""Tensor-parallel causal attention layer (RoPE) for 8 Trainium2 NeuronCores.

Problem: nn_AttentionTier (B=4, T=2048, D=1024, H=16, Dh=64), fp32 I/O.

Sharding: DP=4 over batch x TP=2 over heads (8 heads per core).
  core c -> batch c//2, head group c%2 (heads 8*(c%2) .. 8*(c%2)+8).

v5.1 — v4's bf16 compute (fp8 DoubleRow projections measured slower on
HW: LDWEIGHTS doesn't register as PE activity, so the HAM clock-gate
throttles through the low-duty DR chains) plus:
  - single-descriptor 3D input DMAs and a two-queue preamble so the
    first matmul starts ~4x earlier.
  - global deferred-work queue: projection chains of block tb+1 AND the
    out-projection of block qb-1 drip into attention gaps of block qb
    with stride pacing, so the PE stays fed through the last block.
  - emission-before-use enforced via ensure() keys (same-engine program
    order is a dependency order).
  - ACT runs only exp + the rowsum recip (ln/exp); every PSUM
    evacuation is on DVE; rope cos-mult and rope-add on GPSIMD.

Per core:
  - Attention: per k-chunk the two heads of a plane get score matmuls on
    disjoint PE row groups (tile_position (0,0)/(64,0), concurrent); one
    exp per chunk covers both heads; ones-augmented V gives row sums.
  - Softmax: rowsums gathered to partitions {0,32,64,96}, one ln+exp(-x)
    pass, reciprocal broadcast via K=1 matmuls on packed row groups.
  - Out-projection partials sharded by TOKEN half; pairwise bf16
    ReduceScatter; stored bf16 and upcast to fp32 on the host.
"""

import sys

sys.path.insert(0, "/opt/trn_rl_repo")

import numpy as np

B, T, D = 4, 2048, 1024
H, Dh = 16, 64
N_CORES = 8
P = 128
TB = 512          # token block (matmul moving dim)
NTB = T // TB     # 4
NCC = D // P      # 8 contraction chunks
HLOC = H // 2     # heads per core

_CACHE = {}


def _patch_act_tables():
    """Force every ACT function we use into one table set so bacc emits a
    single hoisted InstLoadActFuncSet instead of thrashing between the
    exp- and ln-anchored sets on every softmax row."""
    import functools
    import concourse.mybir as mybir
    from concourse import bacc, hw_specs

    if getattr(bacc.get_activation_tables, "_attn_patched", False):
        return
    orig = hw_specs.get_activation_tables
    AF = mybir.ActivationFunctionType
    ours = {AF.Exp, AF.Ln, AF.Copy, AF.Identity}

    @functools.cache
    def patched(module_arch):
        tabs = dict(orig(module_arch))
        return {
            name: (fns if name == "natural_log_exp_and_others"
                   else set(fns) - ours)
            for name, fns in tabs.items()
        }

    patched._attn_patched = True
    bacc.get_activation_tables = patched


def _build_program(reps=1):
    import concourse.bass as bass  # noqa: F401
    import concourse.mybir as mybir
    import concourse.tile as tile
    from concourse import bacc

    _patch_act_tables()

    f32 = mybir.dt.float32
    bf16 = mybir.dt.bfloat16
    fp8 = mybir.dt.float8e4
    AF = mybir.ActivationFunctionType
    DR = mybir.MatmulPerfMode.DoubleRow

    nc = bacc.Bacc("TRN2", target_bir_lowering=False, debug=False,
                   num_devices=N_CORES)

    # ---- DRAM I/O ----
    xT_d = nc.dram_tensor("xT", [D, T], bf16, kind="ExternalInput").ap()
    wqkT_d = nc.dram_tensor("wqkT", [D, D], bf16, kind="ExternalInput").ap()
    wvT_d = nc.dram_tensor("wvT", [D, D // 2], bf16,
                           kind="ExternalInput").ap()
    woutT_d = nc.dram_tensor("woutT", [D // 2, D], bf16,
                             kind="ExternalInput").ap()
    r2T_d = nc.dram_tensor("r2T", [P, P], bf16, kind="ExternalInput").ap()
    cos2_d = nc.dram_tensor("cos2", [P, T], bf16, kind="ExternalInput").ap()
    sin2_d = nc.dram_tensor("sin2", [P, T], bf16, kind="ExternalInput").ap()
    tri_d = nc.dram_tensor("tri", [P, P], bf16, kind="ExternalInput").ap()
    out_d = nc.dram_tensor("out", [D // 2, T], bf16,
                           kind="ExternalOutput").ap()

    groups = [[0, 1], [2, 3], [4, 5], [6, 7]]
    ESC = 0.125 / 1024.0  # 1/sqrt(Dh) / (32*32 W_qk host prescale)

    with tile.TileContext(nc) as tc:
        with tc.tile_pool(name="const", bufs=1) as constp, \
             tc.tile_pool(name="big", bufs=1) as bigp, \
             tc.tile_pool(name="w1", bufs=1) as w1p, \
             tc.tile_pool(name="ph1", bufs=3) as ph1, \
             tc.tile_pool(name="xtp", bufs=2) as xtp, \
             tc.tile_pool(name="att", bufs=6) as attp, \
             tc.tile_pool(name="msc", bufs=2) as mscp, \
             tc.tile_pool(name="aop", bufs=2) as aop, \
             tc.tile_pool(name="dram", bufs=2, space="DRAM") as dramp, \
             tc.tile_pool(name="ps_s", bufs=2, space="PSUM") as ps_s, \
             tc.tile_pool(name="ps_o", bufs=1, space="PSUM") as ps_o, \
             tc.tile_pool(name="ps_x", bufs=2, space="PSUM") as ps_x:

            r2T = constp.tile([P, P], bf16)
            tri = constp.tile([P, P], bf16)
            ones_b = constp.tile([P, P], bf16)
            nc.vector.memset(ones_b[:], 1.0)

            # persistent big tensors (bf16)
            qk = bigp.tile([P, NCC, T], bf16)              # rope'd q^T,k^T
            vbar = bigp.tile([P, T // P, HLOC, Dh + 1], bf16)
            nc.vector.tensor_copy(
                vbar[:, :, :, Dh:Dh + 1],
                ones_b[:, None, :HLOC, None].to_broadcast(
                    [P, T // P, HLOC, 1]))

            wqkT = w1p.tile([P, NCC, D], bf16)
            wvT = w1p.tile([P, NCC, D // 2], bf16)
            woutT = w1p.tile([P, NCC // 2, D], bf16)

            # per-chunk descriptors so transfers spread across the 8
            # parallel DMA hardware queues (a single big descriptor
            # serializes on one queue at ~30 GB/s)
            def load_xT(tb):
                t = xtp.tile([P, NCC, TB], bf16, tag="xT")
                for cc in range(NCC):
                    nc.gpsimd.dma_start(
                        t[:, cc],
                        xT_d[cc * P:(cc + 1) * P, tb * TB:(tb + 1) * TB])
                return t

            # preamble: QK-proj inputs first, on two issue queues
            xTs = {0: load_xT(0)}
            for cc in range(NCC):
                nc.sync.dma_start(wqkT[:, cc], wqkT_d[cc * P:(cc + 1) * P, :])
            nc.sync.dma_start(r2T[:], r2T_d[:])
            nc.sync.dma_start(tri[:], tri_d[:])
            for cc in range(NCC):
                nc.gpsimd.dma_start(wvT[:, cc], wvT_d[cc * P:(cc + 1) * P, :])

            # ---- emission helpers ----
            cstiles = {}

            def cs_tiles(tb):
                if tb not in cstiles:
                    tsl = slice(tb * TB, (tb + 1) * TB)
                    cosb = ph1.tile([P, TB], bf16, tag="cosb",
                                    name=f"cosb_{tb}")
                    sinb = ph1.tile([P, TB], bf16, tag="sinb",
                                    name=f"sinb_{tb}")
                    nc.sync.dma_start(cosb[:], cos2_d[:, tsl])
                    nc.sync.dma_start(sinb[:], sin2_d[:, tsl])
                    cstiles[tb] = (cosb, sinb)
                return cstiles[tb]

            def proj_qk(tb, oc):
                """one 128-feature chunk of QK projection + rope"""
                tsl = slice(tb * TB, (tb + 1) * TB)
                xT = xTs[tb]
                cosb, sinb = cs_tiles(tb)
                qk_ps = ps_x.tile([P, TB], f32, tag="xps",
                                  name=f"qkps_{tb}_{oc}")
                for cc in range(NCC):
                    nc.tensor.matmul(
                        qk_ps[:], wqkT[:, cc, oc * P:(oc + 1) * P],
                        xT[:, cc, :],
                        start=(cc == 0), stop=(cc == NCC - 1))
                raw = ph1.tile([P, TB], bf16, tag="raw",
                               name=f"raw_{tb}_{oc}")
                nc.vector.tensor_copy(raw[:], qk_ps[:])
                rot_ps = ps_x.tile([P, TB], f32, tag="xps",
                                   name=f"rotps_{tb}_{oc}")
                nc.tensor.matmul(rot_ps[:], r2T[:], raw[:],
                                 start=True, stop=True)
                ta = ph1.tile([P, TB], bf16, tag="ta", name=f"ta_{tb}_{oc}")
                nc.vector.tensor_tensor(
                    ta[:], rot_ps[:], sinb[:], mybir.AluOpType.mult)
                tb_ = ph1.tile([P, TB], bf16, tag="tb_",
                               name=f"tb__{tb}_{oc}")
                nc.gpsimd.tensor_tensor(
                    tb_[:], raw[:], cosb[:], mybir.AluOpType.mult)
                nc.gpsimd.tensor_tensor(
                    qk[:, oc, tsl], ta[:], tb_[:], mybir.AluOpType.add)

            def proj_v(tb, ts):
                """one 128-token chunk of the V projection"""
                xT = xTs[tb]
                v_ps = ps_x.tile([P, D // 2], f32, tag="xps",
                                 name=f"vps_{tb}_{ts}")
                for cc in range(NCC):
                    nc.tensor.matmul(
                        v_ps[:], xT[:, cc, ts * P:(ts + 1) * P],
                        wvT[:, cc, :],
                        start=(cc == 0), stop=(cc == NCC - 1))
                tc_idx = tb * (TB // P) + ts
                nc.vector.tensor_copy(
                    vbar[:, tc_idx, :, 0:Dh],
                    v_ps[:].rearrange("p (h d) -> p h d", h=HLOC))

            # ---- deferred-work queue -------------------------------
            # pending maps key -> emission thunk. ensure() force-emits
            # (consumers must be emitted after producers on each engine
            # queue); filler() drip-feeds chains into attention gaps.
            pending = {}
            order = []

            def enq(key, thunk):
                pending[key] = thunk
                order.append(key)

            def ensure(key):
                t = pending.pop(key, None)
                if t is not None:
                    t()

            def pop_one():
                while order:
                    key = order[0]
                    if key not in pending:
                        order.pop(0)
                        continue
                    order.pop(0)
                    pending.pop(key)()
                    return True
                return False

            def flush():
                while pop_one():
                    pass

            qstate = {}

            def attn_state(qb):
                if qb not in qstate:
                    sstack = mscp.tile([P, 2, TB], bf16, tag="sstack",
                                       name=f"sstack_{qb}")
                    rstack = mscp.tile([P, 2, TB], bf16, tag="rstack",
                                       name=f"rstack_{qb}")
                    qstate[qb] = (sstack, rstack, [None] * (HLOC // 2))
                return qstate[qb]

            def attn_hp(qb, hp, pace=None, fill=True):
                """attention for head pair hp of q block qb + rowsum
                gather; deferred chains drip in between chunks, paced by a
                fractional credit so the qb-cycle inventory spreads evenly
                over all its chunks."""
                sstack, rstack, osbs = attn_state(qb)
                qsl = slice(qb * TB, (qb + 1) * TB)
                h1, h2 = 2 * hp, 2 * hp + 1
                kpl = NCC // 2 + hp
                qpl = hp
                ensure(("qk", qb, qpl))
                ensure(("qk", qb, kpl))

                def filler():
                    if pace is None:
                        return
                    pace["credit"] += pace["ppc"]
                    while pace["credit"] >= 1.0:
                        pace["credit"] -= 1.0
                        if not pop_one():
                            pace["credit"] = 0.0
                            break

                def smm(s_pair, kc, fsl):
                    ks = slice(kc * P, (kc + 1) * P)
                    nc.tensor.matmul(
                        s_pair[:, 0, fsl],
                        qk[0:Dh, kpl, ks], qk[0:Dh, qpl, qsl][:, fsl],
                        start=True, stop=True, tile_position=(0, 0))
                    nc.tensor.matmul(
                        s_pair[:, 1, fsl],
                        qk[Dh:P, kpl, ks], qk[Dh:P, qpl, qsl][:, fsl],
                        start=True, stop=True, tile_position=(64, 0))

                o_pair = ps_o.tile([Dh + 1, 2, TB], f32, tag="ops",
                                   name=f"op_{qb}_{hp}")
                for kc in range(4 * qb):
                    ensure(("v", kc // 4, kc % 4))
                    s_pair = ps_s.tile([P, 2, TB], f32, tag="sps",
                                       name=f"sp_{qb}_{hp}_{kc}")
                    smm(s_pair, kc, slice(0, TB))
                    pt = attp.tile([P, 2, TB], bf16, tag="pt",
                                   name=f"pt_{qb}_{hp}_{kc}")
                    nc.scalar.activation(
                        pt[:], s_pair[:], AF.Exp, scale=0.125)
                    for j, h in ((0, h1), (1, h2)):
                        nc.tensor.matmul(
                            o_pair[:, j, :], vbar[:, kc, h, :], pt[:, j, :],
                            start=(kc == 0), stop=False,
                            skip_group_check=True)
                    if fill:
                        filler()
                for cr in range(4):
                    kc = 4 * qb + cr
                    ensure(("v", kc // 4, kc % 4))
                    qo = cr * P
                    fsl = slice(qo, TB)
                    s_pair = ps_s.tile([P, 2, TB], f32, tag="sps",
                                       name=f"spd_{qb}_{hp}_{cr}")
                    smm(s_pair, kc, fsl)
                    pt = attp.tile([P, 2, TB], bf16, tag="pt",
                                   name=f"ptd_{qb}_{hp}_{cr}")
                    nc.scalar.activation(
                        pt[:, :, fsl], s_pair[:, :, fsl], AF.Exp,
                        scale=0.125)
                    nc.vector.tensor_tensor(
                        pt[:, :, qo:qo + P], pt[:, :, qo:qo + P],
                        tri[:, None, :].to_broadcast([P, 2, P]),
                        mybir.AluOpType.mult)
                    for j, h in ((0, h1), (1, h2)):
                        nc.tensor.matmul(
                            o_pair[:, j, fsl], vbar[:, kc, h, :],
                            pt[:, j, fsl],
                            start=(kc == 0), stop=(cr == 3),
                            skip_group_check=True)
                    if fill:
                        filler()

                # evacuate PSUM; one DMA gathers both heads' rowsum rows to
                # partition 32*hp of sstack
                osb = mscp.tile([Dh + 1, 2, TB], bf16, tag=f"osb{hp % 2}",
                                name=f"osb_{qb}_{hp}")
                nc.vector.tensor_copy(osb[:], o_pair[:])
                nc.gpsimd.dma_start(
                    sstack[32 * hp:32 * hp + 1, :, :], osb[Dh:Dh + 1, :, :])
                osbs[hp] = osb

            def norm_recip(qb):
                """batched reciprocal of all 8 rowsums: 1/x = exp(-ln x)"""
                sstack, rstack, _ = attn_state(qb)
                nc.scalar.activation(sstack[:], sstack[:], AF.Ln)
                nc.scalar.activation(rstack[:], sstack[:], AF.Exp,
                                     scale=-1.0)

            def norm_apply(qb, aout):
                _, rstack, osbs = attn_state(qb)
                for h in range(HLOC):
                    hp, j = h // 2, h % 2
                    rp = 32 * hp
                    b_ps = ps_x.tile([Dh, TB], f32, tag="xps",
                                     name=f"bps_{qb}_{h}")
                    nc.tensor.matmul(
                        b_ps[:], ones_b[rp:rp + 1, 0:Dh],
                        rstack[rp:rp + 1, j, :],
                        start=True, stop=True, tile_position=(rp, 0))
                    nc.vector.tensor_tensor(
                        aout[64 * j:64 * j + Dh, hp, :],
                        osbs[hp][0:Dh, j, :], b_ps[:],
                        mybir.AluOpType.mult)

            def enq_outproj(qb, aout):
                """queue the out-projection of block qb as per-feature-chunk
                chains; each group's pairwise ReduceScatter fires with its
                4th chain. Core at pair-rank r ends with global output
                features r*512..r*512+512 for all tokens."""
                parts = {}
                for g in (0, 1):
                    parts[g] = dramp.tile([D // 2, TB], bf16, tag="part",
                                          name=f"part_{qb}_{g}")

                def ec_chain(g, i, ec):
                    part = parts[g]
                    f_ps = ps_x.tile([P, TB], f32, tag="xps",
                                     name=f"fps_{qb}_{g}_{ec}")
                    for cc in range(NCC // 2):
                        nc.tensor.matmul(
                            f_ps[:], woutT[:, cc, ec * P:(ec + 1) * P],
                            aout[:, cc, :],
                            start=(cc == 0), stop=(cc == NCC // 2 - 1))
                    fsb = mscp.tile([P, TB], bf16, tag="fsb",
                                    name=f"fsb_{qb}_{g}_{ec}")
                    nc.vector.tensor_copy(fsb[:], f_ps[:])
                    row = ((0 if ec < 4 else 256)
                           + (ec - (0 if ec < 4 else 4) - 2 * g) * P)
                    nc.sync.dma_start(part[row:row + P, :], fsb[:])
                    if i == 3:
                        rs_o = dramp.tile([D // 4, TB], bf16, tag="rs_o",
                                          name=f"rso_{qb}_{g}")
                        nc.gpsimd.collective_compute(
                            "ReduceScatter", mybir.AluOpType.add,
                            replica_groups=groups,
                            ins=[part.opt()], outs=[rs_o.opt()])
                        nc.sync.dma_start(
                            out_d[g * (D // 4):(g + 1) * (D // 4),
                                  qb * TB:(qb + 1) * TB], rs_o[:])

                for g, ecs in ((0, (0, 1, 4, 5)), (1, (2, 3, 6, 7))):
                    for i, ec in enumerate(ecs):
                        enq(("op", qb, g, ec),
                            lambda g=g, i=i, ec=ec: ec_chain(g, i, ec))

            # ---- software-pipelined emission ----
            # block tb=0 projections up front; thereafter projections of
            # tb+1 and the out-projection of qb-1 drip into qb's attention.
            cs_tiles(0)
            for oc in (0, 4, 1, 5, 2, 6, 3, 7):
                proj_qk(0, oc)
            for ts in range(TB // P):
                proj_v(0, ts)
            xTs[1] = load_xT(1)

            # woutT queued after x(1) — not needed until the first out-proj
            for cc in range(NCC // 2):
                nc.sync.dma_start(
                    woutT[:, cc], woutT_d[cc * P:(cc + 1) * P, :])

            attn_hp(0, 0, fill=False)
            for qb in range(NTB):
                if qb + 1 < NTB:
                    ntb = qb + 1
                    for hp in range(HLOC // 2):
                        enq(("qk", ntb, hp),
                            lambda t=ntb, o=hp: proj_qk(t, o))
                        enq(("qk", ntb, NCC // 2 + hp),
                            lambda t=ntb, o=NCC // 2 + hp: proj_qk(t, o))
                        if hp == 0:
                            for ts in range(TB // P):
                                enq(("v", ntb, ts),
                                    lambda t=ntb, s=ts: proj_v(t, s))
                    if qb + 2 < NTB:
                        xTs[qb + 2] = load_xT(qb + 2)
                chunks = 3 * (4 * qb + 4)
                if qb + 1 < NTB:
                    chunks += 4 * (qb + 1) + 4
                pace = {"ppc": len(pending) / max(1, chunks), "credit": 0.0}
                for hp in range(1, HLOC // 2):
                    attn_hp(qb, hp, pace=pace)
                norm_recip(qb)
                # keep the PE fed across the recip latency window
                pop_one()
                pop_one()
                if qb + 1 < NTB:
                    attn_hp(qb + 1, 0, pace=pace)
                # bound deferral: qb-1's out-proj chains must be emitted
                # before qb's are queued (aout pool is 2 deep)
                if qb > 0:
                    for g, ecs in ((0, (0, 1, 4, 5)), (1, (2, 3, 6, 7))):
                        for ec in ecs:
                            ensure(("op", qb - 1, g, ec))
                aout = aop.tile([P, NCC // 2, TB], bf16, tag="aout",
                                name=f"aout_{qb}")
                norm_apply(qb, aout)
                enq_outproj(qb, aout)
                if qb == NTB - 1:
                    flush()
            flush()

    nc.compile()
    return nc


def _host_inputs(x, W_qkv, W_out):
    """Per-core input dicts."""
    import ml_dtypes
    bf = ml_dtypes.bfloat16
    f8 = ml_dtypes.float8_e4m3
    x = np.ascontiguousarray(np.asarray(x, dtype=np.float32))
    W_qkv = np.asarray(W_qkv, dtype=np.float32)
    W_out = np.asarray(W_out, dtype=np.float32)

    def to8(a):
        return np.clip(a, -240.0, 240.0).astype(f8)

    # rope tables, transposed layout, 2-head stack
    inv = 1.0 / (10000.0 ** (np.arange(0, Dh, 2, dtype=np.float64) / Dh))
    ang = np.outer(np.arange(T, dtype=np.float64), inv)        # (T, 32)
    emb = np.concatenate([ang, ang], axis=1)                   # (T, 64)
    cosT = np.cos(emb).astype(np.float32).T                    # (64, T)
    sinT = np.sin(emb).astype(np.float32).T
    cos2 = np.ascontiguousarray(
        np.concatenate([cosT, cosT], 0)).astype(bf)            # (128, T)
    sin2 = np.ascontiguousarray(
        np.concatenate([sinT, sinT], 0)).astype(bf)

    # rotation matrix: rot(q) = R @ q ; lhsT = R2.T
    R = np.zeros((Dh, Dh), np.float32)
    for d in range(Dh // 2):
        R[d, d + Dh // 2] = -1.0
        R[d + Dh // 2, d] = 1.0
    R2 = np.zeros((P, P), np.float32)
    R2[:Dh, :Dh] = R
    R2[Dh:, Dh:] = R
    r2T = np.ascontiguousarray(R2.T).astype(bf)

    # triangular mask in scores^T layout: keep k <= q
    tri = np.triu(np.ones((P, P), np.float32)).astype(bf)

    ins = []
    for c in range(N_CORES):
        b, tp = c // 2, c % 2
        heads = range(8 * tp, 8 * tp + 8)
        wq = np.concatenate([W_qkv[64 * h: 64 * h + 64] for h in heads], 0)
        wk = np.concatenate(
            [W_qkv[D + 64 * h: D + 64 * h + 64] for h in heads], 0)
        wv = np.concatenate(
            [W_qkv[2 * D + 64 * h: 2 * D + 64 * h + 64] for h in heads], 0)
        wqkT = np.ascontiguousarray(
            np.concatenate([wq, wk], 0).T).astype(bf)               # (1024,1024)
        wvT = np.ascontiguousarray(wv.T).astype(bf)                 # (1024,512)
        woutT = np.ascontiguousarray(
            W_out[:, 512 * tp: 512 * tp + 512].T).astype(bf)        # (512,1024)
        ins.append({
            "xT": np.ascontiguousarray(x[b].T).astype(bf),
            "wqkT": wqkT, "wvT": wvT, "woutT": woutT,
            "r2T": r2T, "cos2": cos2, "sin2": sin2, "tri": tri,
        })
    return ins


def kernel(x, W_qkv, W_out):
    import time
    from concourse.bass_utils import run_bass_kernel_spmd

    if "nc" not in _CACHE:
        _CACHE["nc"] = _build_program()
    nc = _CACHE["nc"]
    ins = _host_inputs(x, W_qkv, W_out)
    res = None
    for attempt in range(3):
        try:
            res = run_bass_kernel_spmd(nc, ins, list(range(N_CORES)))
            break
        except Exception:
            # if a previous process' device teardown raced our startup the
            # first execution can die; give the worker time to come back and
            # drop any broken backend handles before retrying
            if attempt == 2:
                raise
            time.sleep(30)
            try:
                import jax
                jax.clear_caches()
                jax.clear_backends()
            except Exception:
                pass
    out = np.empty((B, T, D), dtype=np.float32)
    for c in range(N_CORES):
        b, tp = c // 2, c % 2
        o = np.asarray(res.results[c]["out"]).astype(np.float32)  # (D//2, T)
        out[b, :, 512 * tp: 512 * tp + 512] = o.T
    return out


# revision 23
# speedup vs baseline: 1.1760x; 1.0146x over previous
"""Tensor-parallel causal attention layer (RoPE) for 8 Trainium2 NeuronCores.

Problem: nn_AttentionTier (B=4, T=2048, D=1024, H=16, Dh=64), fp32 I/O.

Sharding: DP=4 over batch x TP=2 over heads (8 heads per core).
  core c -> batch c//2, head group c%2 (heads 8*(c%2) .. 8*(c%2)+8).

v5.1 — v4's bf16 compute (fp8 DoubleRow projections measured slower on
HW: LDWEIGHTS doesn't register as PE activity, so the HAM clock-gate
throttles through the low-duty DR chains) plus:
  - single-descriptor 3D input DMAs and a two-queue preamble so the
    first matmul starts ~4x earlier.
  - global deferred-work queue: projection chains of block tb+1 AND the
    out-projection of block qb-1 drip into attention gaps of block qb
    with stride pacing, so the PE stays fed through the last block.
  - emission-before-use enforced via ensure() keys (same-engine program
    order is a dependency order).
  - ACT runs only exp + the rowsum recip (ln/exp); every PSUM
    evacuation is on DVE; rope cos-mult and rope-add on GPSIMD.

Per core:
  - Attention: per k-chunk the two heads of a plane get score matmuls on
    disjoint PE row groups (tile_position (0,0)/(64,0), concurrent); one
    exp per chunk covers both heads; ones-augmented V gives row sums.
  - Softmax: rowsums gathered to partitions {0,32,64,96}, one ln+exp(-x)
    pass, reciprocal broadcast via K=1 matmuls on packed row groups.
  - Out-projection partials sharded by TOKEN half; pairwise bf16
    ReduceScatter; stored bf16 and upcast to fp32 on the host.
"""

import sys

sys.path.insert(0, "/opt/trn_rl_repo")

import numpy as np

B, T, D = 4, 2048, 1024
H, Dh = 16, 64
N_CORES = 8
P = 128
TB = 512          # token block (matmul moving dim)
NTB = T // TB     # 4
NCC = D // P      # 8 contraction chunks
HLOC = H // 2     # heads per core

_CACHE = {}


def _patch_act_tables():
    """Force every ACT function we use into one table set so bacc emits a
    single hoisted InstLoadActFuncSet instead of thrashing between the
    exp- and ln-anchored sets on every softmax row."""
    import functools
    import concourse.mybir as mybir
    from concourse import bacc, hw_specs

    if getattr(bacc.get_activation_tables, "_attn_patched", False):
        return
    orig = hw_specs.get_activation_tables
    AF = mybir.ActivationFunctionType
    ours = {AF.Exp, AF.Ln, AF.Copy, AF.Identity}

    @functools.cache
    def patched(module_arch):
        tabs = dict(orig(module_arch))
        return {
            name: (fns if name == "natural_log_exp_and_others"
                   else set(fns) - ours)
            for name, fns in tabs.items()
        }

    patched._attn_patched = True
    bacc.get_activation_tables = patched


def _build_program(reps=1):
    import concourse.bass as bass  # noqa: F401
    import concourse.mybir as mybir
    import concourse.tile as tile
    from concourse import bacc

    _patch_act_tables()

    f32 = mybir.dt.float32
    bf16 = mybir.dt.bfloat16
    fp8 = mybir.dt.float8e4
    AF = mybir.ActivationFunctionType
    DR = mybir.MatmulPerfMode.DoubleRow

    nc = bacc.Bacc("TRN2", target_bir_lowering=False, debug=False,
                   num_devices=N_CORES)

    # ---- DRAM I/O ----
    xT_d = nc.dram_tensor("xT", [D, T], bf16, kind="ExternalInput").ap()
    wqkT_d = nc.dram_tensor("wqkT", [D, D], bf16, kind="ExternalInput").ap()
    wvT_d = nc.dram_tensor("wvT", [D, D // 2], bf16,
                           kind="ExternalInput").ap()
    woutT_d = nc.dram_tensor("woutT", [D // 2, D], bf16,
                             kind="ExternalInput").ap()
    r2T_d = nc.dram_tensor("r2T", [P, P], bf16, kind="ExternalInput").ap()
    cos2_d = nc.dram_tensor("cos2", [P, T], bf16, kind="ExternalInput").ap()
    sin2_d = nc.dram_tensor("sin2", [P, T], bf16, kind="ExternalInput").ap()
    tri_d = nc.dram_tensor("tri", [P, P], bf16, kind="ExternalInput").ap()
    out_d = nc.dram_tensor("out", [D // 2, T], bf16,
                           kind="ExternalOutput").ap()

    groups = [[0, 1], [2, 3], [4, 5], [6, 7]]
    ESC = 0.125 / 1024.0  # 1/sqrt(Dh) / (32*32 W_qk host prescale)

    with tile.TileContext(nc) as tc:
        with tc.tile_pool(name="const", bufs=1) as constp, \
             tc.tile_pool(name="big", bufs=1) as bigp, \
             tc.tile_pool(name="w1", bufs=1) as w1p, \
             tc.tile_pool(name="ph1", bufs=3) as ph1, \
             tc.tile_pool(name="xtp", bufs=2) as xtp, \
             tc.tile_pool(name="att", bufs=6) as attp, \
             tc.tile_pool(name="msc", bufs=2) as mscp, \
             tc.tile_pool(name="aop", bufs=2) as aop, \
             tc.tile_pool(name="dram", bufs=2, space="DRAM") as dramp, \
             tc.tile_pool(name="ps_s", bufs=2, space="PSUM") as ps_s, \
             tc.tile_pool(name="ps_o", bufs=1, space="PSUM") as ps_o, \
             tc.tile_pool(name="ps_x", bufs=2, space="PSUM") as ps_x:

            r2T = constp.tile([P, P], bf16)
            tri = constp.tile([P, P], bf16)
            ones_b = constp.tile([P, P], bf16)
            nc.vector.memset(ones_b[:], 1.0)

            # persistent big tensors (bf16)
            qk = bigp.tile([P, NCC, T], bf16)              # rope'd q^T,k^T
            vbar = bigp.tile([P, T // P, HLOC, Dh + 1], bf16)
            nc.vector.tensor_copy(
                vbar[:, :, :, Dh:Dh + 1],
                ones_b[:, None, :HLOC, None].to_broadcast(
                    [P, T // P, HLOC, 1]))

            wqkT = w1p.tile([P, NCC, D], bf16)
            wvT = w1p.tile([P, NCC, D // 2], bf16)
            woutT = w1p.tile([P, NCC // 2, D], bf16)

            # per-chunk descriptors so transfers spread across the 8
            # parallel DMA hardware queues (a single big descriptor
            # serializes on one queue at ~30 GB/s)
            def load_xT(tb):
                t = xtp.tile([P, NCC, TB], bf16, tag="xT")
                for cc in range(NCC):
                    nc.gpsimd.dma_start(
                        t[:, cc],
                        xT_d[cc * P:(cc + 1) * P, tb * TB:(tb + 1) * TB])
                return t

            # preamble: QK-proj inputs first, on two issue queues
            xTs = {0: load_xT(0)}
            for cc in range(NCC):
                nc.sync.dma_start(wqkT[:, cc], wqkT_d[cc * P:(cc + 1) * P, :])
            nc.sync.dma_start(r2T[:], r2T_d[:])
            nc.sync.dma_start(tri[:], tri_d[:])
            for cc in range(NCC):
                nc.gpsimd.dma_start(wvT[:, cc], wvT_d[cc * P:(cc + 1) * P, :])

            # ---- emission helpers ----
            cstiles = {}

            def cs_tiles(tb):
                if tb not in cstiles:
                    tsl = slice(tb * TB, (tb + 1) * TB)
                    cosb = ph1.tile([P, TB], bf16, tag="cosb",
                                    name=f"cosb_{tb}")
                    sinb = ph1.tile([P, TB], bf16, tag="sinb",
                                    name=f"sinb_{tb}")
                    nc.sync.dma_start(cosb[:], cos2_d[:, tsl])
                    nc.sync.dma_start(sinb[:], sin2_d[:, tsl])
                    cstiles[tb] = (cosb, sinb)
                return cstiles[tb]

            def proj_qk(tb, oc):
                """one 128-feature chunk of QK projection + rope"""
                tsl = slice(tb * TB, (tb + 1) * TB)
                xT = xTs[tb]
                cosb, sinb = cs_tiles(tb)
                qk_ps = ps_x.tile([P, TB], f32, tag="xps",
                                  name=f"qkps_{tb}_{oc}")
                for cc in range(NCC):
                    nc.tensor.matmul(
                        qk_ps[:], wqkT[:, cc, oc * P:(oc + 1) * P],
                        xT[:, cc, :],
                        start=(cc == 0), stop=(cc == NCC - 1))
                raw = ph1.tile([P, TB], bf16, tag="raw",
                               name=f"raw_{tb}_{oc}")
                nc.vector.tensor_copy(raw[:], qk_ps[:])
                rot_ps = ps_x.tile([P, TB], f32, tag="xps",
                                   name=f"rotps_{tb}_{oc}")
                nc.tensor.matmul(rot_ps[:], r2T[:], raw[:],
                                 start=True, stop=True)
                ta = ph1.tile([P, TB], bf16, tag="ta", name=f"ta_{tb}_{oc}")
                nc.vector.tensor_tensor(
                    ta[:], rot_ps[:], sinb[:], mybir.AluOpType.mult)
                tb_ = ph1.tile([P, TB], bf16, tag="tb_",
                               name=f"tb__{tb}_{oc}")
                nc.gpsimd.tensor_tensor(
                    tb_[:], raw[:], cosb[:], mybir.AluOpType.mult)
                nc.vector.tensor_tensor(
                    qk[:, oc, tsl], ta[:], tb_[:], mybir.AluOpType.add)

            def proj_v(tb, ts):
                """one 128-token chunk of the V projection"""
                xT = xTs[tb]
                v_ps = ps_x.tile([P, D // 2], f32, tag="xps",
                                 name=f"vps_{tb}_{ts}")
                for cc in range(NCC):
                    nc.tensor.matmul(
                        v_ps[:], xT[:, cc, ts * P:(ts + 1) * P],
                        wvT[:, cc, :],
                        start=(cc == 0), stop=(cc == NCC - 1))
                tc_idx = tb * (TB // P) + ts
                nc.vector.tensor_copy(
                    vbar[:, tc_idx, :, 0:Dh],
                    v_ps[:].rearrange("p (h d) -> p h d", h=HLOC))

            # ---- deferred-work queue -------------------------------
            # pending maps key -> emission thunk. ensure() force-emits
            # (consumers must be emitted after producers on each engine
            # queue); filler() drip-feeds chains into attention gaps.
            pending = {}
            order = []

            def enq(key, thunk):
                pending[key] = thunk
                order.append(key)

            def ensure(key):
                t = pending.pop(key, None)
                if t is not None:
                    t()

            def pop_one():
                while order:
                    key = order[0]
                    if key not in pending:
                        order.pop(0)
                        continue
                    order.pop(0)
                    pending.pop(key)()
                    return True
                return False

            def flush():
                while pop_one():
                    pass

            qstate = {}

            def attn_state(qb):
                if qb not in qstate:
                    sstack = mscp.tile([P, 2, TB], bf16, tag="sstack",
                                       name=f"sstack_{qb}")
                    rstack = mscp.tile([P, 2, TB], bf16, tag="rstack",
                                       name=f"rstack_{qb}")
                    qstate[qb] = (sstack, rstack, [None] * (HLOC // 2))
                return qstate[qb]

            def attn_hp(qb, hp, pace=None, fill=True):
                """attention for head pair hp of q block qb + rowsum
                gather; deferred chains drip in between chunks, paced by a
                fractional credit so the qb-cycle inventory spreads evenly
                over all its chunks."""
                sstack, rstack, osbs = attn_state(qb)
                qsl = slice(qb * TB, (qb + 1) * TB)
                h1, h2 = 2 * hp, 2 * hp + 1
                kpl = NCC // 2 + hp
                qpl = hp
                ensure(("qk", qb, qpl))
                ensure(("qk", qb, kpl))

                def filler():
                    if pace is None:
                        return
                    pace["credit"] += pace["ppc"]
                    while pace["credit"] >= 1.0:
                        pace["credit"] -= 1.0
                        if not pop_one():
                            pace["credit"] = 0.0
                            break

                def smm(s_pair, kc, fsl):
                    ks = slice(kc * P, (kc + 1) * P)
                    nc.tensor.matmul(
                        s_pair[:, 0, fsl],
                        qk[0:Dh, kpl, ks], qk[0:Dh, qpl, qsl][:, fsl],
                        start=True, stop=True, tile_position=(0, 0))
                    nc.tensor.matmul(
                        s_pair[:, 1, fsl],
                        qk[Dh:P, kpl, ks], qk[Dh:P, qpl, qsl][:, fsl],
                        start=True, stop=True, tile_position=(64, 0))

                o_pair = ps_o.tile([Dh + 1, 2, TB], f32, tag="ops",
                                   name=f"op_{qb}_{hp}")
                for kc in range(4 * qb):
                    ensure(("v", kc // 4, kc % 4))
                    s_pair = ps_s.tile([P, 2, TB], f32, tag="sps",
                                       name=f"sp_{qb}_{hp}_{kc}")
                    smm(s_pair, kc, slice(0, TB))
                    pt = attp.tile([P, 2, TB], bf16, tag="pt",
                                   name=f"pt_{qb}_{hp}_{kc}")
                    nc.scalar.activation(
                        pt[:], s_pair[:], AF.Exp, scale=0.125)
                    for j, h in ((0, h1), (1, h2)):
                        nc.tensor.matmul(
                            o_pair[:, j, :], vbar[:, kc, h, :], pt[:, j, :],
                            start=(kc == 0), stop=False,
                            skip_group_check=True)
                    if fill:
                        filler()
                for cr in range(4):
                    kc = 4 * qb + cr
                    ensure(("v", kc // 4, kc % 4))
                    qo = cr * P
                    fsl = slice(qo, TB)
                    s_pair = ps_s.tile([P, 2, TB], f32, tag="sps",
                                       name=f"spd_{qb}_{hp}_{cr}")
                    smm(s_pair, kc, fsl)
                    pt = attp.tile([P, 2, TB], bf16, tag="pt",
                                   name=f"ptd_{qb}_{hp}_{cr}")
                    nc.scalar.activation(
                        pt[:, :, fsl], s_pair[:, :, fsl], AF.Exp,
                        scale=0.125)
                    nc.vector.tensor_tensor(
                        pt[:, :, qo:qo + P], pt[:, :, qo:qo + P],
                        tri[:, None, :].to_broadcast([P, 2, P]),
                        mybir.AluOpType.mult)
                    for j, h in ((0, h1), (1, h2)):
                        nc.tensor.matmul(
                            o_pair[:, j, fsl], vbar[:, kc, h, :],
                            pt[:, j, fsl],
                            start=(kc == 0), stop=(cr == 3),
                            skip_group_check=True)
                    if fill:
                        filler()

                # evacuate PSUM; one DMA gathers both heads' rowsum rows to
                # partition 32*hp of sstack
                osb = mscp.tile([Dh + 1, 2, TB], bf16, tag=f"osb{hp % 2}",
                                name=f"osb_{qb}_{hp}")
                nc.vector.tensor_copy(osb[:], o_pair[:])
                nc.gpsimd.dma_start(
                    sstack[32 * hp:32 * hp + 1, :, :], osb[Dh:Dh + 1, :, :])
                osbs[hp] = osb

            def norm_recip(qb):
                """batched reciprocal of all 8 rowsums: 1/x = exp(-ln x)"""
                sstack, rstack, _ = attn_state(qb)
                nc.scalar.activation(sstack[:], sstack[:], AF.Ln)
                nc.scalar.activation(rstack[:], sstack[:], AF.Exp,
                                     scale=-1.0)

            def norm_apply(qb, aout):
                _, rstack, osbs = attn_state(qb)
                for h in range(HLOC):
                    hp, j = h // 2, h % 2
                    rp = 32 * hp
                    b_ps = ps_x.tile([Dh, TB], f32, tag="xps",
                                     name=f"bps_{qb}_{h}")
                    nc.tensor.matmul(
                        b_ps[:], ones_b[rp:rp + 1, 0:Dh],
                        rstack[rp:rp + 1, j, :],
                        start=True, stop=True, tile_position=(rp, 0))
                    nc.vector.tensor_tensor(
                        aout[64 * j:64 * j + Dh, hp, :],
                        osbs[hp][0:Dh, j, :], b_ps[:],
                        mybir.AluOpType.mult)

            def enq_outproj(qb, aout):
                """queue the out-projection of block qb as per-feature-chunk
                chains; each group's pairwise ReduceScatter fires with its
                4th chain. Core at pair-rank r ends with global output
                features r*512..r*512+512 for all tokens."""
                parts = {}
                for g in (0, 1):
                    parts[g] = dramp.tile([D // 2, TB], bf16, tag="part",
                                          name=f"part_{qb}_{g}")

                def ec_chain(g, i, ec):
                    part = parts[g]
                    f_ps = ps_x.tile([P, TB], f32, tag="xps",
                                     name=f"fps_{qb}_{g}_{ec}")
                    for cc in range(NCC // 2):
                        nc.tensor.matmul(
                            f_ps[:], woutT[:, cc, ec * P:(ec + 1) * P],
                            aout[:, cc, :],
                            start=(cc == 0), stop=(cc == NCC // 2 - 1))
                    fsb = mscp.tile([P, TB], bf16, tag="fsb",
                                    name=f"fsb_{qb}_{g}_{ec}")
                    nc.vector.tensor_copy(fsb[:], f_ps[:])
                    row = ((0 if ec < 4 else 256)
                           + (ec - (0 if ec < 4 else 4) - 2 * g) * P)
                    nc.sync.dma_start(part[row:row + P, :], fsb[:])
                    if i == 3:
                        rs_o = dramp.tile([D // 4, TB], bf16, tag="rs_o",
                                          name=f"rso_{qb}_{g}")
                        nc.gpsimd.collective_compute(
                            "ReduceScatter", mybir.AluOpType.add,
                            replica_groups=groups,
                            ins=[part.opt()], outs=[rs_o.opt()])
                        nc.sync.dma_start(
                            out_d[g * (D // 4):(g + 1) * (D // 4),
                                  qb * TB:(qb + 1) * TB], rs_o[:])

                for g, ecs in ((0, (0, 1, 4, 5)), (1, (2, 3, 6, 7))):
                    for i, ec in enumerate(ecs):
                        enq(("op", qb, g, ec),
                            lambda g=g, i=i, ec=ec: ec_chain(g, i, ec))

            # ---- software-pipelined emission ----
            # block tb=0 projections up front; thereafter projections of
            # tb+1 and the out-projection of qb-1 drip into qb's attention.
            cs_tiles(0)
            for oc in (0, 4, 1, 5, 2, 6, 3, 7):
                proj_qk(0, oc)
            for ts in range(TB // P):
                proj_v(0, ts)
            xTs[1] = load_xT(1)

            # woutT queued after x(1) — not needed until the first out-proj
            for cc in range(NCC // 2):
                nc.sync.dma_start(
                    woutT[:, cc], woutT_d[cc * P:(cc + 1) * P, :])

            attn_hp(0, 0, fill=False)
            for qb in range(NTB):
                if qb + 1 < NTB:
                    ntb = qb + 1
                    for hp in range(HLOC // 2):
                        enq(("qk", ntb, hp),
                            lambda t=ntb, o=hp: proj_qk(t, o))
                        enq(("qk", ntb, NCC // 2 + hp),
                            lambda t=ntb, o=NCC // 2 + hp: proj_qk(t, o))
                        if hp == 0:
                            for ts in range(TB // P):
                                enq(("v", ntb, ts),
                                    lambda t=ntb, s=ts: proj_v(t, s))
                    if qb + 2 < NTB:
                        xTs[qb + 2] = load_xT(qb + 2)
                chunks = 3 * (4 * qb + 4)
                if qb + 1 < NTB:
                    chunks += 4 * (qb + 1) + 4
                pace = {"ppc": len(pending) / max(1, chunks), "credit": 0.0}
                for hp in range(1, HLOC // 2):
                    attn_hp(qb, hp, pace=pace)
                norm_recip(qb)
                # keep the PE fed across the recip latency window
                pop_one()
                pop_one()
                if qb + 1 < NTB:
                    attn_hp(qb + 1, 0, pace=pace)
                # bound deferral: qb-1's out-proj chains must be emitted
                # before qb's are queued (aout pool is 2 deep)
                if qb > 0:
                    for g, ecs in ((0, (0, 1, 4, 5)), (1, (2, 3, 6, 7))):
                        for ec in ecs:
                            ensure(("op", qb - 1, g, ec))
                aout = aop.tile([P, NCC // 2, TB], bf16, tag="aout",
                                name=f"aout_{qb}")
                norm_apply(qb, aout)
                enq_outproj(qb, aout)
                if qb == NTB - 1:
                    flush()
            flush()

    nc.compile()
    return nc


def _host_inputs(x, W_qkv, W_out):
    """Per-core input dicts."""
    import ml_dtypes
    bf = ml_dtypes.bfloat16
    f8 = ml_dtypes.float8_e4m3
    x = np.ascontiguousarray(np.asarray(x, dtype=np.float32))
    W_qkv = np.asarray(W_qkv, dtype=np.float32)
    W_out = np.asarray(W_out, dtype=np.float32)

    def to8(a):
        return np.clip(a, -240.0, 240.0).astype(f8)

    # rope tables, transposed layout, 2-head stack
    inv = 1.0 / (10000.0 ** (np.arange(0, Dh, 2, dtype=np.float64) / Dh))
    ang = np.outer(np.arange(T, dtype=np.float64), inv)        # (T, 32)
    emb = np.concatenate([ang, ang], axis=1)                   # (T, 64)
    cosT = np.cos(emb).astype(np.float32).T                    # (64, T)
    sinT = np.sin(emb).astype(np.float32).T
    cos2 = np.ascontiguousarray(
        np.concatenate([cosT, cosT], 0)).astype(bf)            # (128, T)
    sin2 = np.ascontiguousarray(
        np.concatenate([sinT, sinT], 0)).astype(bf)

    # rotation matrix: rot(q) = R @ q ; lhsT = R2.T
    R = np.zeros((Dh, Dh), np.float32)
    for d in range(Dh // 2):
        R[d, d + Dh // 2] = -1.0
        R[d + Dh // 2, d] = 1.0
    R2 = np.zeros((P, P), np.float32)
    R2[:Dh, :Dh] = R
    R2[Dh:, Dh:] = R
    r2T = np.ascontiguousarray(R2.T).astype(bf)

    # triangular mask in scores^T layout: keep k <= q
    tri = np.triu(np.ones((P, P), np.float32)).astype(bf)

    ins = []
    for c in range(N_CORES):
        b, tp = c // 2, c % 2
        heads = range(8 * tp, 8 * tp + 8)
        wq = np.concatenate([W_qkv[64 * h: 64 * h + 64] for h in heads], 0)
        wk = np.concatenate(
            [W_qkv[D + 64 * h: D + 64 * h + 64] for h in heads], 0)
        wv = np.concatenate(
            [W_qkv[2 * D + 64 * h: 2 * D + 64 * h + 64] for h in heads], 0)
        wqkT = np.ascontiguousarray(
            np.concatenate([wq, wk], 0).T).astype(bf)               # (1024,1024)
        wvT = np.ascontiguousarray(wv.T).astype(bf)                 # (1024,512)
        woutT = np.ascontiguousarray(
            W_out[:, 512 * tp: 512 * tp + 512].T).astype(bf)        # (512,1024)
        ins.append({
            "xT": np.ascontiguousarray(x[b].T).astype(bf),
            "wqkT": wqkT, "wvT": wvT, "woutT": woutT,
            "r2T": r2T, "cos2": cos2, "sin2": sin2, "tri": tri,
        })
    return ins


def kernel(x, W_qkv, W_out):
    import time
    from concourse.bass_utils import run_bass_kernel_spmd

    if "nc" not in _CACHE:
        _CACHE["nc"] = _build_program()
    nc = _CACHE["nc"]
    ins = _host_inputs(x, W_qkv, W_out)
    res = None
    for attempt in range(3):
        try:
            res = run_bass_kernel_spmd(nc, ins, list(range(N_CORES)))
            break
        except Exception:
            # if a previous process' device teardown raced our startup the
            # first execution can die; give the worker time to come back and
            # drop any broken backend handles before retrying
            if attempt == 2:
                raise
            time.sleep(30)
            try:
                import jax
                jax.clear_caches()
                jax.clear_backends()
            except Exception:
                pass
    out = np.empty((B, T, D), dtype=np.float32)
    for c in range(N_CORES):
        b, tp = c // 2, c % 2
        o = np.asarray(res.results[c]["out"]).astype(np.float32)  # (D//2, T)
        out[b, :, 512 * tp: 512 * tp + 512] = o.T
    return out
